# revision 47
# baseline (speedup 1.0000x reference)
"""BinarizeConv2dSDP kernel for Trainium2 (8 NeuronCores, data-parallel over batch).

out = conv2d(sign(x), sign(M + sum_k rv[k] * Z[k]), stride 1, pad 1) * Alpha

Key simplification: the reference normalizes (M, Z) by rsqrt(M^2 + sum Z^2 / SCALE)
before forming w = rv@Z + M, but that factor is strictly positive and applied
multiplicatively to the whole expression, so sign(w) is unaffected.  The binary
weights are just sign(M + sum_k rv[k] Z[k]).

Strategy per core (8 images each, measured ~89-92us on hardware, exact output):
  - weight gen on DVE: w = sum rv_k Z_k + M in [oc, ic*9] layout, sign -> bf16,
    then 9 PE transposes (against an anti-diagonal permutation) produce the
    column-reversed, pair-interleaved fp8e4 weight layout that
    DoubleRowSwInterleave expects (plus one all-zero tap so 9 taps = 5 pairs).
  - conv: 5 fp8 DoubleRowSwInterleave matmuls per 8-row chunk (2 taps per
    matmul, K=256 effective contraction), accumulated in PSUM over a 58-wide
    zero-padded sign(x) image; the free dim spans whole padded rows (464) so
    the moving AP stays 3D, leaving 2 garbage columns per row that the
    eviction skips.
  - Alpha scaling on DVE during PSUM->SBUF eviction; steady-state output DMAs
    ride the SWDGE (GpSimd) ring so they never head-of-line block input loads
    on the FIFO HWDGE SP ring (the last two images' outputs switch to the
    then-idle SP ring, whose issue path is cheaper).
"""

import numpy as np
from contextlib import ExitStack

import concourse.bass as bass
import concourse.mybir as mybir
import concourse.tile as tile
from concourse.bacc import Bacc
from concourse.bass_utils import run_bass_kernel_spmd

N_CORES = 8
B, C, H, W = 64, 128, 56, 56
BPC = B // N_CORES  # images per core
KS, K = 3, 5
PH, PW = H + 2, W + 2  # zero-padded image
CHUNK_ROWS = 8
N_CHUNKS = H // CHUNK_ROWS
FREE = CHUNK_ROWS * W  # valid output elements per chunk (448)
FREE_R = CHUNK_ROWS * PW  # matmul free dim incl. garbage cols (464 <= 512)
F32 = mybir.dt.float32
BF16 = mybir.dt.bfloat16
F8 = mybir.dt.float8e4


def build_kernel(rv_vals):
    """Build the single-core Bass module (SPMD: same program on all 8 cores).

    rv_vals: the 5 rv scalars, baked as immediates into the weight-gen ops.
    """
    # Bacc (not plain Bass): its compile() pass pipeline legalizes sync waits
    # (TRN2 allows at most 1 embedded wait per engine instruction; excess waits
    # are split into InstEventSemaphore via generate_event_semaphores).
    nc = Bacc()
    x_p = nc.declare_dram_parameter("x", [BPC, C, H, W], F32, isOutput=False)
    m_p = nc.declare_dram_parameter("M", [C, C, KS, KS], F32, isOutput=False)
    z_p = nc.declare_dram_parameter("Z", [K, C, C, KS, KS], F32, isOutput=False)
    a_p = nc.declare_dram_parameter("Alpha", [C, 1, 1], F32, isOutput=False)
    rv_p = nc.declare_dram_parameter("rv", [1, K], F32, isOutput=False)
    out_p = nc.declare_dram_parameter("out", [BPC, C, H, W], F32, isOutput=True)

    from concourse.masks import make_identity

    NW = C * KS * KS  # 1152 weight elements per out-channel row
    HALF = (H // 2) * W  # first-half image elements (28 rows)

    with tile.TileContext(nc) as tc, ExitStack() as ctx:
        const = ctx.enter_context(tc.tile_pool(name="const", bufs=1))
        wg = ctx.enter_context(tc.tile_pool(name="wg", bufs=1))
        zpool = ctx.enter_context(tc.tile_pool(name="zpool", bufs=1))
        xin = ctx.enter_context(tc.tile_pool(name="xin", bufs=4))
        pad = ctx.enter_context(tc.tile_pool(name="pad", bufs=4))
        opool = ctx.enter_context(tc.tile_pool(name="opool", bufs=3))
        ps_t = ctx.enter_context(tc.tile_pool(name="ps_t", bufs=2, space="PSUM"))
        ps_c = ctx.enter_context(tc.tile_pool(name="ps_c", bufs=6, space="PSUM"))

        # ---- constants ----
        # Anti-diagonal permutation: transpose against it yields the transposed
        # tap with REVERSED out-channel columns, which is exactly the column
        # order DoubleRowSwInterleave's weight layout wants.
        identity = const.tile([C, C], BF16)
        nc.gpsimd.memset(identity[:], 0.0)
        nc.gpsimd.affine_select(
            out=identity[:],
            in_=identity[:],
            compare_op=mybir.AluOpType.not_equal,
            fill=1.0,
            base=-(C - 1),
            pattern=[[1, C]],
            channel_multiplier=1,
        )
        # Alpha/rv ride the ACT HWDGE ring: each HWDGE issue costs the owning
        # sequencer ~0.6-0.9us, and the SP ring's first slots belong to x0.
        alpha_sb = const.tile([C, 1], F32)
        nc.scalar.dma_start(alpha_sb[:], a_p[:].rearrange("c a b -> c (a b)"))
        rv_sb = const.tile([1, K], F32)
        nc.scalar.dma_start(rv_sb[:], rv_p[:])

        x_ap = x_p[:]
        o_ap = out_p[:]

        # The HWDGE SP ring drains FIFO, so the issue order below is the wire
        # order: a 10-row sliver of image 0 first (just enough for chunk 0's
        # sign, which is all that gates the first conv matmul), then the
        # weight tensors (they own the wire -- the first matmul can't start
        # without them), then the rest of image 0 in two pieces sized so each
        # sign lands ahead of the PE reaching its chunks.
        X0P = [0, 10 * W, 28 * W, H * W]  # piece boundaries (elements)
        x_sb_0 = xin.tile([C, H * W], F32, tag="x_sb")
        nc.sync.dma_start(
            x_sb_0[:, X0P[0] : X0P[1]],
            x_ap[0].rearrange("c h w -> c (h w)")[:, X0P[0] : X0P[1]],
        )
        z_sbs = []
        for k in range(K):
            z_sbs.append(zpool.tile([C, NW], F32, name=f"z{k}", tag=f"z{k}"))
            nc.sync.dma_start(
                z_sbs[k][:], z_p[k].rearrange("o i a b -> o (i a b)")
            )
        m_sb = wg.tile([C, NW], F32)
        nc.sync.dma_start(m_sb[:], m_p[:].rearrange("o i a b -> o (i a b)"))
        for pi in (1, 2):
            nc.sync.dma_start(
                x_sb_0[:, X0P[pi] : X0P[pi + 1]],
                x_ap[0].rearrange("c h w -> c (h w)")[:, X0P[pi] : X0P[pi + 1]],
            )

        # ---- weight generation: w = (sum_k rv_k Z_k) + M ----
        w_sb = wg.tile([C, NW], F32)
        bw_sb = wg.tile([C, NW], BF16)
        nc.vector.tensor_scalar_mul(w_sb[:], z_sbs[0][:], float(rv_vals[0]))
        for k in range(1, K):
            nc.vector.scalar_tensor_tensor(
                w_sb[:],
                z_sbs[k][:],
                float(rv_vals[k]),
                w_sb[:],
                mybir.AluOpType.mult,
                mybir.AluOpType.add,
            )
        nc.vector.tensor_add(w_sb[:], w_sb[:], m_sb[:])
        nc.scalar.sign(bw_sb[:], w_sb[:])

        # Transpose each tap's [oc, ic] into [ic, oc-reversed] (via the
        # anti-diagonal permutation), then interleave tap pairs column-wise as
        # fp8e4 (+-1 exact): [A127 B127 A126 B126 ... A0 B0] per partition --
        # the DoubleRowSwInterleave weight layout.  Pre-interleaving makes
        # LDWEIGHTS a single 128-column pass (~107ns, hides under the matmul)
        # instead of DoubleRow's 256-column reload.  Tap 9 (pair 4, slot B)
        # stays all-zero.
        wt = const.tile([C, 5, 2 * C], F8)
        nc.vector.memset(wt[:, 4, :], 0.0)
        bw_r = bw_sb[:].rearrange("o (i j) -> o i j", j=KS * KS)
        for j in range(KS * KS):
            tp = ps_t.tile([C, C], BF16)
            nc.tensor.transpose(tp[:], bw_r[:, :, j], identity[:])
            pair, slot = divmod(j, 2)
            wt_h = wt[:].tensor
            dst = bass.AP(wt_h, pair * 2 * C + slot, [[5 * 2 * C, C], [2, C]])
            nc.vector.tensor_copy(dst, tp[:])
        # rv reaches the kernel as baked immediates; touch the tensor so the
        # bound input isn't dead.
        nc.vector.tensor_copy(w_sb[0:1, 0:K], rv_sb[0:1, :])

        def tap_off(r0, j):
            # flat offset of (out-row r0, tap j)'s top-left read in the padded image
            if j == KS * KS:  # zero tap: alias tap 8's window (weights are 0)
                j = KS * KS - 1
            return (r0 + j // KS) * PW + (j % KS)

        def load_sign(i):
            """Image load + binarize into a fresh zero-bordered pad tile."""
            if i == 0:
                x_sb = x_sb_0
            else:
                x_sb = xin.tile([C, H * W], F32, tag="x_sb")
                nc.sync.dma_start(x_sb[:], x_ap[i].rearrange("c h w -> c (h w)"))
            ba = pad.tile([C, PH * PW + 2], F8, tag="ba")
            ba_r = ba[:, 0 : PH * PW].rearrange("c (h w) -> c h w", w=PW)
            # Zero only the pad border (sign() fills the interior).
            nc.vector.memset(ba[:, 0:PW], 0.0)
            nc.vector.memset(ba[:, (PH - 1) * PW : PH * PW + 2], 0.0)
            nc.vector.memset(ba_r[:, 1 : H + 1, 0:1], 0.0)
            nc.vector.memset(ba_r[:, 1 : H + 1, W + 1 : PW], 0.0)
            x_r = x_sb[:].rearrange("c (h w) -> c h w", w=W)
            if i == 0:
                # piecewise signs matching the X0P DMA pieces, so each runs as
                # soon as its slice of the image lands
                for pi in range(3):
                    r_lo, r_hi = X0P[pi] // W, X0P[pi + 1] // W
                    nc.scalar.sign(
                        ba_r[:, 1 + r_lo : 1 + r_hi, 1 : W + 1], x_r[:, r_lo:r_hi]
                    )
            else:
                nc.scalar.sign(ba_r[:, 1 : H + 1, 1 : W + 1], x_r)
            return ba

        def conv_store(i, ba):
            """9-tap binary conv via 5 DoubleRow matmuls per chunk + eviction."""
            o_sb = opool.tile([C, H * W], F32, tag="o_sb")
            for ch in range(N_CHUNKS):
                pt = ps_c.tile([C, FREE_R], F32, tag="pt")
                r0 = ch * CHUNK_ROWS
                for p in range(5):
                    o0 = tap_off(r0, 2 * p)
                    o1 = tap_off(r0, 2 * p + 1)
                    rhs = bass.AP(
                        ba[:].tensor,
                        o0,
                        [[PH * PW + 2, C], [o1 - o0, 2], [1, FREE_R]],
                    )
                    nc.tensor.matmul(
                        pt[:],
                        wt[:, p, :],
                        rhs,
                        start=(p == 0),
                        stop=(p == 4),
                        perf_mode=mybir.MatmulPerfMode.DoubleRowSwInterleave,
                    )
                # PSUM -> SBUF eviction with per-channel Alpha scale on DVE,
                # skipping the 2 garbage columns per row.
                nc.vector.tensor_scalar_mul(
                    o_sb[:, ch * FREE : (ch + 1) * FREE].rearrange(
                        "c (a b) -> c a b", b=W
                    ),
                    pt[:].rearrange("c (a b) -> c a b", b=PW)[:, :, 0:W],
                    alpha_sb[:, 0:1],
                )
                # Output DMAs ride the SWDGE (GpSimd) ring: an output DMA
                # waiting on evictions would head-of-line block later input
                # loads on the FIFO HWDGE ring.  Half-image granularity
                # shrinks the end-of-kernel tail.
                # Images >= 6 finish after every input load has drained, so
                # their outputs can ride the idle SP HWDGE ring (cheaper
                # issue, no head-of-line risk anymore).
                out_dma = nc.sync.dma_start if i >= 6 else nc.gpsimd.dma_start
                last_img = i == BPC - 1
                if ch == 3:
                    out_dma(
                        o_ap[i].rearrange("c h w -> c (h w)")[:, 0 : 4 * FREE],
                        o_sb[:, 0 : 4 * FREE],
                    )
                elif ch == N_CHUNKS - 1 and not last_img:
                    out_dma(
                        o_ap[i].rearrange("c h w -> c (h w)")[:, 4 * FREE :],
                        o_sb[:, 4 * FREE :],
                    )
                elif last_img and ch > 3:
                    # per-chunk pieces at the very end shrink the output tail
                    out_dma(
                        o_ap[i].rearrange("c h w -> c (h w)")[
                            :, ch * FREE : (ch + 1) * FREE
                        ],
                        o_sb[:, ch * FREE : (ch + 1) * FREE],
                    )

        # Software-pipelined: image i+1's load/sign issues before image i's
        # conv+store so ScalarE signs (and input DMAs) always run ahead.
        prev_ba = None
        for i in range(BPC):
            ba = load_sign(i)
            if prev_ba is not None:
                conv_store(i - 1, prev_ba)
            prev_ba = ba
        conv_store(BPC - 1, prev_ba)

    nc.finalize()
    return nc


_CACHE = {}


def _get_nc(rv):
    key = rv.tobytes()
    if key not in _CACHE:
        _CACHE[key] = build_kernel(np.asarray(rv, np.float32).reshape(-1))
    return _CACHE[key]


def _run(inputs, trace=False):
    x = np.ascontiguousarray(np.asarray(inputs["x"], np.float32))
    M = np.ascontiguousarray(np.asarray(inputs["M"], np.float32))
    Z = np.ascontiguousarray(np.asarray(inputs["Z"], np.float32))
    Alpha = np.ascontiguousarray(np.asarray(inputs["Alpha"], np.float32))
    rv = np.ascontiguousarray(np.asarray(inputs["rv"], np.float32))
    nc = _get_nc(rv)
    in_maps = [
        {"x": x[c * BPC : (c + 1) * BPC], "M": M, "Z": Z, "Alpha": Alpha, "rv": rv}
        for c in range(N_CORES)
    ]
    res = run_bass_kernel_spmd(nc, in_maps, list(range(N_CORES)), trace=trace)
    out = np.concatenate([res.results[c]["out"] for c in range(N_CORES)], axis=0)
    return out, res


def kernel(**inputs):
    out, _ = _run(inputs, trace=False)
    return out


def kernel_traced(**inputs):
    out, res = _run(inputs, trace=True)
    return out, res


# revision 48
# speedup vs baseline: 1.1139x; 1.1139x over previous
"""BinarizeConv2dSDP kernel for Trainium2 (8 NeuronCores, data-parallel over batch).

out = conv2d(sign(x), sign(M + sum_k rv[k] * Z[k]), stride 1, pad 1) * Alpha

Key simplification: the reference normalizes (M, Z) by rsqrt(M^2 + sum Z^2 / SCALE)
before forming w = rv@Z + M, but that factor is strictly positive and applied
multiplicatively to the whole expression, so sign(w) is unaffected.  The binary
weights are just sign(M + sum_k rv[k] Z[k]).

Strategy per core (8 images each, measured ~89-92us on hardware, exact output):
  - weight gen on DVE: w = sum rv_k Z_k + M in [oc, ic*9] layout, sign -> bf16,
    then 9 PE transposes (against an anti-diagonal permutation) produce the
    column-reversed, pair-interleaved fp8e4 weight layout that
    DoubleRowSwInterleave expects (plus one all-zero tap so 9 taps = 5 pairs).
  - conv: 5 fp8 DoubleRowSwInterleave matmuls per 8-row chunk (2 taps per
    matmul, K=256 effective contraction), accumulated in PSUM over a 58-wide
    zero-padded sign(x) image; the free dim spans whole padded rows (464) so
    the moving AP stays 3D, leaving 2 garbage columns per row that the
    eviction skips.
  - Alpha scaling on DVE during PSUM->SBUF eviction; steady-state output DMAs
    ride the SWDGE (GpSimd) ring so they never head-of-line block input loads
    on the FIFO HWDGE SP ring (the last two images' outputs switch to the
    then-idle SP ring, whose issue path is cheaper).
"""

import numpy as np
from contextlib import ExitStack

import concourse.bass as bass
import concourse.mybir as mybir
import concourse.tile as tile
from concourse.bacc import Bacc
from concourse.bass_utils import run_bass_kernel_spmd

N_CORES = 8
B, C, H, W = 64, 128, 56, 56
BPC = B // N_CORES  # images per core
KS, K = 3, 5
PH, PW = H + 2, W + 2  # zero-padded image
CHUNK_ROWS = 8
N_CHUNKS = H // CHUNK_ROWS
FREE = CHUNK_ROWS * W  # valid output elements per chunk (448)
FREE_R = CHUNK_ROWS * PW  # matmul free dim incl. garbage cols (464 <= 512)
F32 = mybir.dt.float32
BF16 = mybir.dt.bfloat16
F8 = mybir.dt.float8e4


def build_kernel(rv_vals):
    """Build the single-core Bass module (SPMD: same program on all 8 cores).

    rv_vals: the 5 rv scalars, baked as immediates into the weight-gen ops.
    """
    # Bacc (not plain Bass): its compile() pass pipeline legalizes sync waits
    # (TRN2 allows at most 1 embedded wait per engine instruction; excess waits
    # are split into InstEventSemaphore via generate_event_semaphores).
    nc = Bacc()
    x_p = nc.declare_dram_parameter("x", [BPC, C, H, W], F32, isOutput=False)
    m_p = nc.declare_dram_parameter("M", [C, C, KS, KS], F32, isOutput=False)
    z_p = nc.declare_dram_parameter("Z", [K, C, C, KS, KS], F32, isOutput=False)
    a_p = nc.declare_dram_parameter("Alpha", [C, 1, 1], F32, isOutput=False)
    rv_p = nc.declare_dram_parameter("rv", [1, K], F32, isOutput=False)
    out_p = nc.declare_dram_parameter("out", [BPC, C, H, W], F32, isOutput=True)

    from concourse.masks import make_identity

    NW = C * KS * KS  # 1152 weight elements per out-channel row
    HALF = (H // 2) * W  # first-half image elements (28 rows)

    with tile.TileContext(nc) as tc, ExitStack() as ctx:
        const = ctx.enter_context(tc.tile_pool(name="const", bufs=1))
        wg = ctx.enter_context(tc.tile_pool(name="wg", bufs=1))
        zpool = ctx.enter_context(tc.tile_pool(name="zpool", bufs=1))
        xin = ctx.enter_context(tc.tile_pool(name="xin", bufs=4))
        pad = ctx.enter_context(tc.tile_pool(name="pad", bufs=4))
        opool = ctx.enter_context(tc.tile_pool(name="opool", bufs=3))
        ps_t = ctx.enter_context(tc.tile_pool(name="ps_t", bufs=2, space="PSUM"))
        ps_c = ctx.enter_context(tc.tile_pool(name="ps_c", bufs=6, space="PSUM"))

        # ---- constants ----
        # Anti-diagonal permutation: transpose against it yields the transposed
        # tap with REVERSED out-channel columns, which is exactly the column
        # order DoubleRowSwInterleave's weight layout wants.
        identity = const.tile([C, C], BF16)
        nc.gpsimd.memset(identity[:], 0.0)
        nc.gpsimd.affine_select(
            out=identity[:],
            in_=identity[:],
            compare_op=mybir.AluOpType.not_equal,
            fill=1.0,
            base=-(C - 1),
            pattern=[[1, C]],
            channel_multiplier=1,
        )
        # Alpha/rv ride the ACT HWDGE ring: each HWDGE issue costs the owning
        # sequencer ~0.6-0.9us, and the SP ring's first slots belong to x0.
        alpha_sb = const.tile([C, 1], F32)
        nc.scalar.dma_start(alpha_sb[:], a_p[:].rearrange("c a b -> c (a b)"))
        rv_sb = const.tile([1, K], F32)
        nc.scalar.dma_start(rv_sb[:], rv_p[:])

        x_ap = x_p[:]
        o_ap = out_p[:]

        # The HWDGE SP ring drains FIFO, so the issue order below is the wire
        # order: half of image 0 first (so its sign() overlaps weight-gen),
        # then the Z tensors (the accumulation chain consumes them in order),
        # then M (only needed by the final add), then the rest of image 0.
        x_sb_0 = xin.tile([C, H * W], F32, tag="x_sb")
        nc.sync.dma_start(
            x_sb_0[:, 0:HALF], x_ap[0].rearrange("c h w -> c (h w)")[:, 0:HALF]
        )
        z_sbs = []
        for k in range(K):
            z_sbs.append(zpool.tile([C, NW], F32, name=f"z{k}", tag=f"z{k}"))
            nc.sync.dma_start(
                z_sbs[k][:], z_p[k].rearrange("o i a b -> o (i a b)")
            )
        m_sb = wg.tile([C, NW], F32)
        nc.sync.dma_start(m_sb[:], m_p[:].rearrange("o i a b -> o (i a b)"))
        nc.sync.dma_start(
            x_sb_0[:, HALF:], x_ap[0].rearrange("c h w -> c (h w)")[:, HALF:]
        )

        # ---- weight generation: w = (sum_k rv_k Z_k) + M ----
        w_sb = wg.tile([C, NW], F32)
        bw_sb = wg.tile([C, NW], BF16)
        nc.vector.tensor_scalar_mul(w_sb[:], z_sbs[0][:], float(rv_vals[0]))
        for k in range(1, K):
            nc.vector.scalar_tensor_tensor(
                w_sb[:],
                z_sbs[k][:],
                float(rv_vals[k]),
                w_sb[:],
                mybir.AluOpType.mult,
                mybir.AluOpType.add,
            )
        nc.vector.tensor_add(w_sb[:], w_sb[:], m_sb[:])
        nc.scalar.sign(bw_sb[:], w_sb[:])

        # Transpose each tap's [oc, ic] into [ic, oc-reversed] (via the
        # anti-diagonal permutation), then interleave tap pairs column-wise as
        # fp8e4 (+-1 exact): [A127 B127 A126 B126 ... A0 B0] per partition --
        # the DoubleRowSwInterleave weight layout.  Pre-interleaving makes
        # LDWEIGHTS a single 128-column pass (~107ns, hides under the matmul)
        # instead of DoubleRow's 256-column reload.  Tap 9 (pair 4, slot B)
        # stays all-zero.
        wt = const.tile([C, 5, 2 * C], F8)
        nc.vector.memset(wt[:, 4, :], 0.0)
        bw_r = bw_sb[:].rearrange("o (i j) -> o i j", j=KS * KS)
        for j in range(KS * KS):
            tp = ps_t.tile([C, C], BF16)
            nc.tensor.transpose(tp[:], bw_r[:, :, j], identity[:])
            pair, slot = divmod(j, 2)
            wt_h = wt[:].tensor
            dst = bass.AP(wt_h, pair * 2 * C + slot, [[5 * 2 * C, C], [2, C]])
            nc.vector.tensor_copy(dst, tp[:])
        # rv reaches the kernel as baked immediates; touch the tensor so the
        # bound input isn't dead.
        nc.vector.tensor_copy(w_sb[0:1, 0:K], rv_sb[0:1, :])

        def tap_off(r0, j):
            # flat offset of (out-row r0, tap j)'s top-left read in the padded image
            if j == KS * KS:  # zero tap: alias tap 8's window (weights are 0)
                j = KS * KS - 1
            return (r0 + j // KS) * PW + (j % KS)

        def load_sign(i):
            """Image load + binarize into a fresh zero-bordered pad tile."""
            if i == 0:
                x_sb = x_sb_0
            else:
                x_sb = xin.tile([C, H * W], F32, tag="x_sb")
                nc.sync.dma_start(x_sb[:], x_ap[i].rearrange("c h w -> c (h w)"))
            ba = pad.tile([C, PH * PW + 2], F8, tag="ba")
            ba_r = ba[:, 0 : PH * PW].rearrange("c (h w) -> c h w", w=PW)
            # Zero only the pad border (sign() fills the interior).
            nc.vector.memset(ba[:, 0:PW], 0.0)
            nc.vector.memset(ba[:, (PH - 1) * PW : PH * PW + 2], 0.0)
            nc.vector.memset(ba_r[:, 1 : H + 1, 0:1], 0.0)
            nc.vector.memset(ba_r[:, 1 : H + 1, W + 1 : PW], 0.0)
            x_r = x_sb[:].rearrange("c (h w) -> c h w", w=W)
            if i == 0:
                # two signs so the first half runs as soon as its DMA lands
                nc.scalar.sign(ba_r[:, 1 : H // 2 + 1, 1 : W + 1], x_r[:, : H // 2])
                nc.scalar.sign(ba_r[:, H // 2 + 1 : H + 1, 1 : W + 1], x_r[:, H // 2 :])
            else:
                nc.scalar.sign(ba_r[:, 1 : H + 1, 1 : W + 1], x_r)
            return ba

        def conv_store(i, ba):
            """9-tap binary conv via 5 DoubleRow matmuls per chunk + eviction."""
            o_sb = opool.tile([C, H * W], F32, tag="o_sb")
            for ch in range(N_CHUNKS):
                pt = ps_c.tile([C, FREE_R], F32, tag="pt")
                r0 = ch * CHUNK_ROWS
                for p in range(5):
                    o0 = tap_off(r0, 2 * p)
                    o1 = tap_off(r0, 2 * p + 1)
                    rhs = bass.AP(
                        ba[:].tensor,
                        o0,
                        [[PH * PW + 2, C], [o1 - o0, 2], [1, FREE_R]],
                    )
                    nc.tensor.matmul(
                        pt[:],
                        wt[:, p, :],
                        rhs,
                        start=(p == 0),
                        stop=(p == 4),
                        perf_mode=mybir.MatmulPerfMode.DoubleRowSwInterleave,
                    )
                # PSUM -> SBUF eviction with per-channel Alpha scale on DVE,
                # skipping the 2 garbage columns per row.
                nc.vector.tensor_scalar_mul(
                    o_sb[:, ch * FREE : (ch + 1) * FREE].rearrange(
                        "c (a b) -> c a b", b=W
                    ),
                    pt[:].rearrange("c (a b) -> c a b", b=PW)[:, :, 0:W],
                    alpha_sb[:, 0:1],
                )
                # Output DMAs ride the SWDGE (GpSimd) ring: an output DMA
                # waiting on evictions would head-of-line block later input
                # loads on the FIFO HWDGE ring.  Half-image granularity
                # shrinks the end-of-kernel tail.
                # Images >= 6 finish after every input load has drained, so
                # their outputs can ride the idle SP HWDGE ring (cheaper
                # issue, no head-of-line risk anymore).
                out_dma = nc.sync.dma_start if i >= 6 else nc.gpsimd.dma_start
                last_img = i == BPC - 1
                if ch == 3:
                    out_dma(
                        o_ap[i].rearrange("c h w -> c (h w)")[:, 0 : 4 * FREE],
                        o_sb[:, 0 : 4 * FREE],
                    )
                elif ch == N_CHUNKS - 1 and not last_img:
                    out_dma(
                        o_ap[i].rearrange("c h w -> c (h w)")[:, 4 * FREE :],
                        o_sb[:, 4 * FREE :],
                    )
                elif last_img and ch > 3:
                    # per-chunk pieces at the very end shrink the output tail
                    out_dma(
                        o_ap[i].rearrange("c h w -> c (h w)")[
                            :, ch * FREE : (ch + 1) * FREE
                        ],
                        o_sb[:, ch * FREE : (ch + 1) * FREE],
                    )

        # Software-pipelined: image i+1's load/sign issues before image i's
        # conv+store so ScalarE signs (and input DMAs) always run ahead.
        prev_ba = None
        for i in range(BPC):
            ba = load_sign(i)
            if prev_ba is not None:
                conv_store(i - 1, prev_ba)
            prev_ba = ba
        conv_store(BPC - 1, prev_ba)

    nc.finalize()
    return nc


_CACHE = {}


def _get_nc(rv):
    key = rv.tobytes()
    if key not in _CACHE:
        _CACHE[key] = build_kernel(np.asarray(rv, np.float32).reshape(-1))
    return _CACHE[key]


def _run(inputs, trace=False):
    x = np.ascontiguousarray(np.asarray(inputs["x"], np.float32))
    M = np.ascontiguousarray(np.asarray(inputs["M"], np.float32))
    Z = np.ascontiguousarray(np.asarray(inputs["Z"], np.float32))
    Alpha = np.ascontiguousarray(np.asarray(inputs["Alpha"], np.float32))
    rv = np.ascontiguousarray(np.asarray(inputs["rv"], np.float32))
    nc = _get_nc(rv)
    in_maps = [
        {"x": x[c * BPC : (c + 1) * BPC], "M": M, "Z": Z, "Alpha": Alpha, "rv": rv}
        for c in range(N_CORES)
    ]
    res = run_bass_kernel_spmd(nc, in_maps, list(range(N_CORES)), trace=trace)
    out = np.concatenate([res.results[c]["out"] for c in range(N_CORES)], axis=0)
    return out, res


def kernel(**inputs):
    out, _ = _run(inputs, trace=False)
    return out


def kernel_traced(**inputs):
    out, res = _run(inputs, trace=True)
    return out, res
